# revision 38
# baseline (speedup 1.0000x reference)
"""Causal self-attention (GQA + RoPE) Trainium2 Bass kernel.

Sharding: 8 cores = batch(2) x kv-group(4). Each core computes its batch's
4 q-heads / 1 kv-head and a row-shard of the Wo projection; the 4 partial
outputs per batch are summed on host (all-reduce replacement).

Fused single-pass pipeline over 512-query rows: for each row n we
project+RoPE x block n, immediately run the causal attention row jq=n
(which only needs k/v blocks 0..4n+3, all available), normalize, and run
the Wo projection + y writeout for the row's 4 token blocks.

Engine budget per core (measured): PE ~120us is the critical engine, so
everything movable is pushed off it and off its feeders:
- exp acts fused across the head pair (one ACTIVATE per k-block).
- softmax denominators via DVE reciprocal_approx_fast (not Act Ln/Exp).
- O normalized straight out of PSUM (no o65 staging copy).
- y written PSUM->DRAM by DMA (no DVE copy, no SBUF staging).
- DMA split across both HW DGE queues (sync + scalar) with kv-first
  weight layout so the first projection unblocks after ~0.8MB.
"""

import numpy as np

import concourse.bass as bass
import concourse.mybir as mybir
from concourse.tile import TileContext
from concourse.bass_utils import run_bass_kernel_spmd

F32 = mybir.dt.float32
F32R = mybir.dt.float32r
BF16 = mybir.dt.bfloat16

B, T, C = 2, 2048, 1024
H, HKV, D = 16, 4, 64
HALF = D // 2  # 32
GQ = H // HKV  # 4 q heads per group
FQ = GQ * D    # 256 q features per group
NT = T // 512  # 4 row blocks of 512
KT = C // 128  # 8 contraction tiles
MT = 3         # m-tiles: 0=kv(128), 1=q01(128), 2=q23(128)


def _split_excess_waits(nc, max_waits=1):
    """walrus here encodes at most one sync-wait per instruction; hoist the
    rest into standalone EventSemaphore instructions (raw-bass encoding)."""
    n = 0
    for fn in nc.m.functions:
        for bb in fn.blocks:
            new = []
            changed = False
            for inst in bb.instructions:
                si = inst.sync_info
                if si is not None and len(si.on_wait) > max_waits:
                    waits = list(si.on_wait)
                    for j, w in enumerate(waits[max_waits:]):
                        ev = mybir.InstEventSemaphore(
                            name=f"{inst.name}-ws{j}",
                            engine=inst.engine,
                            ins=[],
                            outs=[],
                            sync_info=mybir.SyncInfo(on_wait=[w], on_update=[]),
                        )
                        new.append(ev)
                        n += 1
                    inst.sync_info = mybir.SyncInfo(
                        on_wait=waits[:max_waits], on_update=list(si.on_update)
                    )
                    changed = True
                new.append(inst)
            if changed:
                bb.instructions = new
    return n


def _build():
    nc = bass.Bass()
    xt_d = nc.dram_tensor("xt", [C, T], BF16, kind="ExternalInput")
    wproj_d = nc.dram_tensor("wproj", [128, MT, KT, 128], BF16, kind="ExternalInput")
    wo_d = nc.dram_tensor("wo", [FQ, C], BF16, kind="ExternalInput")
    atab_d = nc.dram_tensor("atab", [128, T], BF16, kind="ExternalInput")
    btab_d = nc.dram_tensor("btab", [128, T], BF16, kind="ExternalInput")
    trib_d = nc.dram_tensor("trib", [128, 128], BF16, kind="ExternalInput")
    identr_d = nc.dram_tensor("identr", [64, 64], BF16, kind="ExternalInput")
    rotp_d = nc.dram_tensor("rotp", [128, 128], BF16, kind="ExternalInput")
    y_d = nc.dram_tensor("y", [T, C], BF16, kind="ExternalOutput")

    xt_r = xt_d.rearrange("(ko p) t -> p ko t", p=128)

    with TileContext(nc) as tc:
        from contextlib import ExitStack

        with ExitStack() as ctx:
            const = ctx.enter_context(tc.tile_pool(name="const", bufs=1))
            pers = ctx.enter_context(tc.tile_pool(name="pers", bufs=1))
            # --- constants ---
            wproj_sb = const.tile([128, MT, KT, 128], BF16)
            wo_sb = const.tile([128, 2, C], BF16)
            atab = const.tile([128, T], BF16)
            btab = const.tile([128, T], BF16)
            trib = const.tile([128, 128], BF16)
            identr = const.tile([128, 64], BF16)
            rotp = const.tile([128, 128], BF16)

            # --- persistent activations ---
            qr = [pers.tile([128, T], BF16, name=f"qr{i}") for i in range(2)]
            # k^T zero-padded to 128 contraction rows: kr0 = [k; 0] for even
            # heads, kr1 = [0; k] for odd heads -> S matmuls engage the full
            # PE array while the zero half kills the other head's q rows.
            kr0 = pers.tile([128, T], BF16)
            kr1 = pers.tile([128, T], BF16)
            vsb = pers.tile([128, T // 128, 128], BF16)  # v natural + ones cols
            opk = [pers.tile([128, T], BF16, name=f"opk{i}") for i in range(2)]  # per-pair normalized O^T for Wo

            xpool = ctx.enter_context(tc.tile_pool(name="xp", bufs=2))
            tmp = ctx.enter_context(tc.tile_pool(name="tmp", bufs=2))
            ppool = ctx.enter_context(tc.tile_pool(name="pp", bufs=7))
            rcpool = ctx.enter_context(tc.tile_pool(name="rc", bufs=2))
            ypool = ctx.enter_context(tc.tile_pool(name="yp", bufs=6))
            # PSUM: pp(2) + s(2x2) + o(1x2) = 8 banks
            pp_ps = ctx.enter_context(
                tc.tile_pool(name="ppps", bufs=2, space="PSUM")
            )
            spool = ctx.enter_context(
                tc.tile_pool(name="sps", bufs=2, space="PSUM")
            )
            opool = ctx.enter_context(
                tc.tile_pool(name="ops", bufs=1, space="PSUM")
            )

            xrows = {}
            yq = [0]  # alternator for y store queue

            def emit_x_dma(n, engs=(nc.sync, nc.sync)):
                xr = xpool.tile([128, KT, 512], BF16, tag="x", name=f"x{n}")
                xrows[n] = xr
                for half in range(2):
                    ks = slice(4 * half, 4 * half + 4)
                    engs[half].dma_start(
                        xr[:, ks], xt_r[:, ks, bass.ts(n, 512)]
                    )

            def emit_preamble_dma():
                xr = xpool.tile([128, KT, 512], BF16, tag="x", name="x0")
                xrows[0] = xr
                # critical path: proj(0) kv gates on wproj m=0 + x row 0;
                # 2-ktile x chunks let the k-loop start on the first chunk;
                # each queue's order matches first-use order downstream.
                nc.sync.dma_start(wproj_sb[:, 0, 0:4], wproj_d[:, 0, 0:4])
                nc.sync.dma_start(xr[:, 0:2], xt_r[:, 0:2, bass.ts(0, 512)])
                nc.scalar.dma_start(xr[:, 4:6], xt_r[:, 4:6, bass.ts(0, 512)])
                nc.sync.dma_start(wproj_sb[:, 0, 4:8], wproj_d[:, 0, 4:8])
                nc.sync.dma_start(xr[:, 2:4], xt_r[:, 2:4, bass.ts(0, 512)])
                nc.scalar.dma_start(xr[:, 6:8], xt_r[:, 6:8, bass.ts(0, 512)])
                nc.scalar.dma_start(wproj_sb[:, 1], wproj_d[:, 1])
                nc.scalar.dma_start(atab[:, 0:512], atab_d[:, 0:512])
                nc.sync.dma_start(btab[:, 0:512], btab_d[:, 0:512])
                nc.sync.dma_start(rotp[:], rotp_d[:])
                nc.scalar.dma_start(wproj_sb[:, 2], wproj_d[:, 2])
                nc.sync.dma_start(identr[64:128, :], identr_d[:])
                nc.sync.dma_start(trib[:], trib_d[:])
                # zero pads + ones column via gpsimd (keeps DMA queues free)
                nc.gpsimd.memset(kr0[64:128, :], 0.0)
                nc.gpsimd.memset(kr1[0:64, :], 0.0)
                nc.gpsimd.memset(vsb[:, :, 64:128], 1.0)

            def emit_late_consts():
                # issued after proj(0) kv+q01 so row 0's critical chain isn't
                # queued behind these bulk transfers
                emit_x_dma(1, engs=(nc.sync, nc.scalar))
                nc.sync.dma_start(atab[:, 512:T], atab_d[:, 512:T])
                nc.scalar.dma_start(btab[:, 512:T], btab_d[:, 512:T])
                wo_r = wo_d.rearrange("(ko p) c -> p ko c", p=128)
                nc.sync.dma_start(wo_sb[:, 0:1], wo_r[:, 0:1])
                nc.scalar.dma_start(wo_sb[:, 1:2], wo_r[:, 1:2])

            def emit_proj_a(n, m, st):
                """projection phase A for token block n, m-tile m: matmuls,
                PSUM->SBUF copy, rotate-half via PE permutation matmul
                (no DMA on the rope critical path), cos mult.
                m=0: kv (k rope rows 0:64, v transpose), m=1: q01, m=2: q23."""
                xr = xrows[n]
                ps = pp_ps.tile([128, 512], F32, tag="pp", name=f"ps{m}")
                for k in range(KT):
                    nc.tensor.matmul(
                        ps[:],
                        wproj_sb[:, m, k, :],
                        xr[:, k],
                        start=(k == 0),
                        stop=(k == KT - 1),
                    )
                rows = 64 if m == 0 else 128
                qt_t = tmp.tile([128, 512], BF16, tag="qt", name="qt", bufs=3)
                plain = qt_t[:]
                nc.vector.tensor_copy(plain, ps[:])
                qs_ps = pp_ps.tile([128, 512], F32, tag="pp", name=f"qs{m}")
                nc.tensor.matmul(
                    qs_ps[:], rotp[:], plain, start=True, stop=True
                )
                t1 = tmp.tile([128, 512], BF16, tag="t1")
                nc.vector.tensor_tensor(
                    t1[0:rows],
                    plain[0:rows],
                    atab[0:rows, bass.ts(n, 512)],
                    mybir.AluOpType.mult,
                )
                st.update(plain=plain, qs=qs_ps, t1=t1, rows=rows)

            def emit_proj_b(n, m, st):
                """projection phase B: sin mult (reads the permuted copy from
                PSUM) and the rope add."""
                qs, t1, rows = st["qs"], st["t1"], st["rows"]
                t2 = tmp.tile([128, 512], BF16, tag="t2")
                nc.vector.tensor_tensor(
                    t2[0:rows],
                    qs[0:rows],
                    btab[0:rows, bass.ts(n, 512)],
                    mybir.AluOpType.mult,
                )
                dest = kr0 if m == 0 else qr[m - 1]
                nc.vector.tensor_tensor(
                    dest[0:rows, bass.ts(n, 512)],
                    t1[0:rows],
                    t2[0:rows],
                    mybir.AluOpType.add,
                )

            def emit_proj_x(n, m, st):
                """kv extras: kr1 duplicate + v transposes (not needed
                until the first O matmul, so they trail the rope core)."""
                if m != 0:
                    return
                plain = st["plain"]
                nc.vector.tensor_copy(
                    kr1[64:128, bass.ts(n, 512)],
                    kr0[0:64, bass.ts(n, 512)],
                )
                for tt in range(4 * n, 4 * n + 4):
                    vt_ps = spool.tile([128, 64], BF16, tag="s", name="vt")
                    nc.tensor.transpose(
                        vt_ps[:],
                        plain[64:128, bass.ts(tt - 4 * n, 128)],
                        identr[64:128, :],
                    )
                    nc.vector.tensor_copy(vsb[:, tt, 0:64], vt_ps[:])

            def emit_attn_pair(n, hp, fillers=None, pre_norm=None, reserve=3):
                """S/exp/O for one head pair of query row n. After each
                k-block, one filler closure (prev row's Wo pieces / future
                projections) is emitted as ready-to-run PE work to absorb
                exp stalls."""
                fillers = fillers if fillers is not None else []
                jq = n
                nkb = 4 * (jq + 1)
                qtile = qr[hp]
                o_ps = opool.tile([128, 2, 512], F32, tag="o", name=f"o{hp}")
                pends = []

                def emit_o(pnd):
                    kb, col0, p_sb = pnd
                    for hh in range(2):
                        nc.tensor.matmul(
                            o_ps[:, hh, col0:512],
                            vsb[:, kb, :],
                            p_sb[:, hh, col0:512],
                            start=(kb == 0),
                            stop=(kb == nkb - 1),
                        )

                for kb in range(nkb):
                    j = kb - 4 * jq
                    col0 = max(j, 0) * 128
                    s_ps = spool.tile([128, 2, 512], F32, tag="s", name="s")
                    for hh in range(2):
                        krt = kr0 if hh == 0 else kr1
                        nc.tensor.matmul(
                            s_ps[:, hh, col0:512],
                            krt[:, bass.ts(kb, 128)],
                            qtile[:, jq * 512 + col0 : jq * 512 + 512],
                            start=True,
                            stop=True,
                        )
                    p_sb = ppool.tile(
                        [128, 2, 512], BF16, tag="p", name="pb", bufs=7
                    )
                    nc.scalar.activation(
                        p_sb[:, :, col0:512],
                        s_ps[:, :, col0:512],
                        mybir.ActivationFunctionType.Exp,
                        scale=0.125,
                    )
                    if j >= 0:
                        for hh in range(2):
                            nc.gpsimd.tensor_tensor(
                                p_sb[:, hh, col0 : col0 + 128],
                                p_sb[:, hh, col0 : col0 + 128],
                                trib[:],
                                mybir.AluOpType.mult,
                            )
                    # O matmuls run a few k-blocks behind S (p bufs=6) so the
                    # PE queue never reaches an O whose exp hasn't drained.
                    if len(pends) == 4:
                        emit_o(pends.pop(0))
                    pends.append((kb, col0, p_sb))
                    # the deferred previous norm is emitted a few k-blocks in
                    # so its rowsum acts trail this pair's first exps; filler
                    # pops wait for it (they read the opk tile it writes).
                    if pre_norm is not None and kb >= 2:
                        pre_norm()
                        pre_norm = None
                    elif pre_norm is None and len(fillers) > reserve:
                        fillers.pop(0)()
                for pnd in pends:
                    emit_o(pnd)
                if pre_norm is not None:
                    pre_norm()
                # numerators to SBUF (frees the o bank early); rows 64:128
                # hold the rowsum already broadcast across 64 partitions
                # (ones columns of vsb); 1/rowsum via Ln + Exp(-x) on the
                # scalar engine (same act table as the softmax Exp), with Ln
                # reading PSUM directly so it overlaps the staging copy.
                o_sb = rcpool.tile(
                    [64, 2, 512], F32, tag="osb", name=f"osb{jq}_{hp}", bufs=2
                )
                nc.vector.tensor_copy(o_sb[:], o_ps[0:64, :, :])
                lnd = rcpool.tile(
                    [64, 2, 512], F32, tag="lnd", name=f"ln{jq}_{hp}", bufs=2
                )
                nc.scalar.activation(
                    lnd[:], o_ps[64:128, :, :],
                    mybir.ActivationFunctionType.Ln,
                )
                rc = rcpool.tile(
                    [64, 2, 512], F32, tag="rc", name=f"rc{jq}_{hp}", bufs=2
                )
                nc.scalar.activation(
                    rc[:], lnd[:],
                    mybir.ActivationFunctionType.Exp,
                    scale=-1.0,
                )
                return o_sb, rc

            def emit_norm_pair(n, hp, o_sb, rc):
                """apply 1/rowsum via DVE mult -> opk (bf16), all-SBUF."""
                jq = n
                for hh in range(2):
                    nc.vector.tensor_tensor(
                        opk[hp][hh * 64 : hh * 64 + 64, bass.ts(jq, 512)],
                        o_sb[0:64, hh, :],
                        rc[:, hh, :],
                        mybir.AluOpType.mult,
                    )

            def wo_fillers(n, tail_from=None):
                """Wo projection + PSUM->DRAM writeout for row n's 4 token
                blocks, as 8 closures interleaved into the next row's
                attention stream as PE bubble-fill. Pieces from `tail_from`
                onward draw their PSUM bank from the (by then free) o pool
                and copy on the idle Act engine: used for the last row's
                reserved pieces, which run during the final rowsum chain."""
                out = []
                ys = {}

                def piece(t, nn, tailish):
                    def emit():
                        if tailish:
                            wps = opool.tile(
                                [128, 512], F32, tag="o", name="wpso"
                            )
                        else:
                            wps = pp_ps.tile(
                                [128, 512], F32, tag="pp", name="wps"
                            )
                        for k in range(2):
                            nc.tensor.matmul(
                                wps[:],
                                opk[k][:, bass.ts(t, 128)],
                                wo_sb[:, k, bass.ts(nn, 512)],
                                start=(k == 0),
                                stop=(k == 1),
                            )
                        if nn == 0:
                            ys[t] = ypool.tile(
                                [128, C], BF16, tag="y", name="ysb", bufs=4
                            )
                        y_sb = ys[t]
                        if tailish and (yq[0] & 1):
                            nc.scalar.activation(
                                y_sb[:, bass.ts(nn, 512)], wps[:],
                                mybir.ActivationFunctionType.Copy,
                            )
                        else:
                            nc.vector.tensor_copy(
                                y_sb[:, bass.ts(nn, 512)], wps[:]
                            )
                        if tailish:
                            yq[0] += 1
                        if nn == 1:
                            eng = (nc.sync, nc.scalar)[t & 1]
                            eng.dma_start(y_d[bass.ts(t, 128), :], y_sb[:])

                    return emit

                i = 0
                for t in range(4 * n, 4 * n + 4):
                    for nn in range(2):
                        out.append(
                            piece(t, nn, tail_from is not None and i >= tail_from)
                        )
                        i += 1
                return out

            def proj_fillers(n, ms=(0, 1, 2)):
                """A/B/X phases as separate closures, A's leading by one
                slot, so the swap DMA latency hides between filler pops."""
                out = []
                pend = []
                for m in ms:
                    st = {}
                    out.append(
                        (lambda nn, mm, s: lambda: emit_proj_a(nn, mm, s))(n, m, st)
                    )
                    if pend:
                        out.append(pend.pop(0))
                    pend.append(
                        (lambda nn, mm, s: lambda: emit_proj_b(nn, mm, s))(n, m, st)
                    )
                    if m == 0:
                        pend.append(
                            (lambda nn, s: lambda: emit_proj_x(nn, 0, s))(n, st)
                        )
                out.extend(pend)
                return out

            # ---- schedule ----
            # dummy act up front so the act-table load (1.3us) runs during
            # the DMA-bound startup instead of before the first softmax exp
            scr = const.tile([1, 8], F32)
            nc.gpsimd.memset(scr[0:1, 0:4], 1.0)
            nc.scalar.activation(
                scr[0:1, 4:8], scr[0:1, 0:4],
                mybir.ActivationFunctionType.Exp,
            )
            emit_preamble_dma()
            st00, st01 = {}, {}
            emit_proj_a(0, 0, st00)  # kv
            emit_proj_a(0, 1, st01)  # q01
            emit_proj_b(0, 0, st00)
            emit_proj_b(0, 1, st01)
            emit_proj_x(0, 0, st00)
            emit_late_consts()
            # row 0: pair 0 interleaved with the remaining projections
            f0 = proj_fillers(0, (2,)) + proj_fillers(1)
            o_sb, rc = emit_attn_pair(0, 0, f0)
            emit_x_dma(2)
            o_sb1, rc1 = emit_attn_pair(
                0, 1, f0,
                pre_norm=(lambda s, r: lambda: emit_norm_pair(0, 0, s, r))(o_sb, rc),
            )
            while f0:
                f0.pop(0)()
            pending = (lambda s, r: lambda: emit_norm_pair(0, 1, s, r))(o_sb1, rc1)

            for n in range(1, NT):
                fill = wo_fillers(n - 1)
                if n == 1:
                    fill += proj_fillers(2)
                elif n == 2:
                    fill += proj_fillers(3)
                rsv = 6 if n == NT - 1 else 3
                o_sb, rc = emit_attn_pair(n, 0, fill, pre_norm=pending, reserve=rsv)
                if n + 2 < NT:
                    emit_x_dma(n + 2)
                o_sb1, rc1 = emit_attn_pair(
                    n, 1, fill,
                    pre_norm=(lambda nn, s, r: lambda: emit_norm_pair(nn, 0, s, r))(n, o_sb, rc),
                    reserve=rsv,
                )
                while fill:
                    fill.pop(0)()
                pending = (lambda nn, s, r: lambda: emit_norm_pair(nn, 1, s, r))(n, o_sb1, rc1)
            # the last pair's norm gates the tail Wo k=1 accumulation
            pending()
            # tail: the last row's Wo. k=0 contractions could start after
            # norm(3,0), but norm(3,1) only trails by the bc+mult chain; keep
            # the k-split interleave so the k=0 half runs during it.
            n3 = NT - 1
            for t in range(4 * n3, 4 * n3 + 4):
                wpair = []
                for nn in range(2):
                    wpool = pp_ps if (t + nn) % 2 == 0 else spool
                    wps = wpool.tile(
                        [128, 512], F32,
                        tag="pp" if wpool is pp_ps else "s",
                        name="wps",
                    )
                    wpair.append(wps)
                    nc.tensor.matmul(
                        wps[:],
                        opk[0][:, bass.ts(t, 128)],
                        wo_sb[:, 0, bass.ts(nn, 512)],
                        start=True,
                        stop=False,
                    )
                y_sb = ypool.tile([128, C], BF16, tag="y", name="ysb", bufs=4)
                for nn in range(2):
                    wps = wpair[nn]
                    nc.tensor.matmul(
                        wps[:],
                        opk[1][:, bass.ts(t, 128)],
                        wo_sb[:, 1, bass.ts(nn, 512)],
                        start=False,
                        stop=True,
                    )
                    if yq[0] & 1:
                        nc.scalar.activation(
                            y_sb[:, bass.ts(nn, 512)], wps[:],
                            mybir.ActivationFunctionType.Copy,
                        )
                    else:
                        nc.vector.tensor_copy(
                            y_sb[:, bass.ts(nn, 512)], wps[:]
                        )
                    yq[0] += 1
                eng = (nc.sync, nc.scalar)[t & 1]
                eng.dma_start(y_d[bass.ts(t, 128), :], y_sb[:])

    _split_excess_waits(nc)
    return nc


_NC_CACHE = None


def _get_nc():
    global _NC_CACHE
    if _NC_CACHE is None:
        _NC_CACHE = _build()
    return _NC_CACHE


def _host_prep(x, cos, sin, Wq, Wk, Wv, Wo):
    import ml_dtypes

    cos2 = np.asarray(cos, np.float32).reshape(T, HALF)  # [T, 32]
    sin2 = np.asarray(sin, np.float32).reshape(T, HALF)
    atab = np.tile(cos2.T, (4, 1)).astype(ml_dtypes.bfloat16)  # [128, T]
    btab = np.tile(np.vstack([sin2.T, -sin2.T]), (2, 1)).astype(
        ml_dtypes.bfloat16
    )
    k_i = np.arange(128)[:, None]
    q_i = np.arange(128)[None, :]
    trib = np.where(k_i > q_i, np.float32(0.0), np.float32(1.0)).astype(
        ml_dtypes.bfloat16
    )
    identr = np.eye(64, dtype=ml_dtypes.bfloat16)
    ii = np.arange(128)
    rotp = (ii[:, None] == (ii[None, :] ^ 32)).astype(ml_dtypes.bfloat16)

    in_maps = []
    for core in range(8):
        b, g = core // 4, core % 4
        xt = np.ascontiguousarray(
            np.asarray(x[b], np.float32).T.astype(ml_dtypes.bfloat16)
        )  # [C, T]
        # f-tiles ordered kv(128), q01(128), q23(128); laid out
        # [p, m, ko, f] so each m-tile loads as one contiguous-run DMA.
        wfull = np.concatenate(
            [
                Wk[:, g * D : (g + 1) * D],
                Wv[:, g * D : (g + 1) * D],
                Wq[:, g * FQ : (g + 1) * FQ],
            ],
            axis=1,
        ).astype(ml_dtypes.bfloat16)  # [C, 384]: kv | q01 | q23
        wproj = np.ascontiguousarray(
            wfull.reshape(KT, 128, MT, 128).transpose(1, 2, 0, 3)
        )  # [128, MT, KT, 128]
        wo = np.ascontiguousarray(
            Wo[g * FQ : (g + 1) * FQ, :].astype(ml_dtypes.bfloat16)
        )
        in_maps.append(
            {
                "xt": xt,
                "wproj": wproj,
                "wo": wo,
                "atab": atab,
                "btab": btab,
                "trib": trib,
                "identr": identr,
                "rotp": rotp,
            }
        )
    return in_maps


def kernel(x, cos, sin, Wq, Wk, Wv, Wo, _want_trace=False, _trace_kwargs=None):
    nc = _get_nc()
    in_maps = _host_prep(x, cos, sin, Wq, Wk, Wv, Wo)
    kw = {}
    if _want_trace:
        kw = dict(trace=True, **(_trace_kwargs or {}))
    res = run_bass_kernel_spmd(nc, in_maps, list(range(8)), **kw)
    y = np.zeros((B, T, C), np.float32)
    for core in range(8):
        b = core // 4
        y[b] += np.asarray(res.results[core]["y"], dtype=np.float32)
    if _want_trace:
        kernel.last_result = res
    return y


# revision 39
# speedup vs baseline: 1.0051x; 1.0051x over previous
"""Causal self-attention (GQA + RoPE) Trainium2 Bass kernel.

Sharding: 8 cores = batch(2) x kv-group(4). Each core computes its batch's
4 q-heads / 1 kv-head and a row-shard of the Wo projection; the 4 partial
outputs per batch are summed on host (all-reduce replacement).

Fused single-pass pipeline over 512-query rows: for each row n we
project+RoPE x block n, immediately run the causal attention row jq=n
(which only needs k/v blocks 0..4n+3, all available), normalize, and run
the Wo projection + y writeout for the row's 4 token blocks.

Engine budget per core (measured): PE ~120us is the critical engine, so
everything movable is pushed off it and off its feeders:
- exp acts fused across the head pair (one ACTIVATE per k-block).
- softmax denominators via DVE reciprocal_approx_fast (not Act Ln/Exp).
- O normalized straight out of PSUM (no o65 staging copy).
- y written PSUM->DRAM by DMA (no DVE copy, no SBUF staging).
- DMA split across both HW DGE queues (sync + scalar) with kv-first
  weight layout so the first projection unblocks after ~0.8MB.
"""

import numpy as np

import concourse.bass as bass
import concourse.mybir as mybir
from concourse.tile import TileContext
from concourse.bass_utils import run_bass_kernel_spmd

F32 = mybir.dt.float32
F32R = mybir.dt.float32r
BF16 = mybir.dt.bfloat16

B, T, C = 2, 2048, 1024
H, HKV, D = 16, 4, 64
HALF = D // 2  # 32
GQ = H // HKV  # 4 q heads per group
FQ = GQ * D    # 256 q features per group
NT = T // 512  # 4 row blocks of 512
KT = C // 128  # 8 contraction tiles
MT = 3         # m-tiles: 0=kv(128), 1=q01(128), 2=q23(128)


def _split_excess_waits(nc, max_waits=1):
    """walrus here encodes at most one sync-wait per instruction; hoist the
    rest into standalone EventSemaphore instructions (raw-bass encoding)."""
    n = 0
    for fn in nc.m.functions:
        for bb in fn.blocks:
            new = []
            changed = False
            for inst in bb.instructions:
                si = inst.sync_info
                if si is not None and len(si.on_wait) > max_waits:
                    waits = list(si.on_wait)
                    for j, w in enumerate(waits[max_waits:]):
                        ev = mybir.InstEventSemaphore(
                            name=f"{inst.name}-ws{j}",
                            engine=inst.engine,
                            ins=[],
                            outs=[],
                            sync_info=mybir.SyncInfo(on_wait=[w], on_update=[]),
                        )
                        new.append(ev)
                        n += 1
                    inst.sync_info = mybir.SyncInfo(
                        on_wait=waits[:max_waits], on_update=list(si.on_update)
                    )
                    changed = True
                new.append(inst)
            if changed:
                bb.instructions = new
    return n


def _build():
    nc = bass.Bass()
    xt_d = nc.dram_tensor("xt", [C, T], BF16, kind="ExternalInput")
    wproj_d = nc.dram_tensor("wproj", [128, MT, KT, 128], BF16, kind="ExternalInput")
    wo_d = nc.dram_tensor("wo", [FQ, C], BF16, kind="ExternalInput")
    atab_d = nc.dram_tensor("atab", [128, T], BF16, kind="ExternalInput")
    btab_d = nc.dram_tensor("btab", [128, T], BF16, kind="ExternalInput")
    trib_d = nc.dram_tensor("trib", [128, 128], BF16, kind="ExternalInput")
    identr_d = nc.dram_tensor("identr", [64, 64], BF16, kind="ExternalInput")
    rotp_d = nc.dram_tensor("rotp", [128, 128], BF16, kind="ExternalInput")
    y_d = nc.dram_tensor("y", [T, C], BF16, kind="ExternalOutput")

    xt_r = xt_d.rearrange("(ko p) t -> p ko t", p=128)

    with TileContext(nc) as tc:
        from contextlib import ExitStack

        with ExitStack() as ctx:
            const = ctx.enter_context(tc.tile_pool(name="const", bufs=1))
            pers = ctx.enter_context(tc.tile_pool(name="pers", bufs=1))
            # --- constants ---
            wproj_sb = const.tile([128, MT, KT, 128], BF16)
            wo_sb = const.tile([128, 2, C], BF16)
            atab = const.tile([128, T], BF16)
            btab = const.tile([128, T], BF16)
            trib = const.tile([128, 128], BF16)
            identr = const.tile([128, 64], BF16)
            rotp = const.tile([128, 128], BF16)

            # --- persistent activations ---
            qr = [pers.tile([128, T], BF16, name=f"qr{i}") for i in range(2)]
            # k^T zero-padded to 128 contraction rows: kr0 = [k; 0] for even
            # heads, kr1 = [0; k] for odd heads -> S matmuls engage the full
            # PE array while the zero half kills the other head's q rows.
            kr0 = pers.tile([128, T], BF16)
            kr1 = pers.tile([128, T], BF16)
            vsb = pers.tile([128, T // 128, 128], BF16)  # v natural + ones cols
            opk = [pers.tile([128, T], BF16, name=f"opk{i}") for i in range(2)]  # per-pair normalized O^T for Wo

            xpool = ctx.enter_context(tc.tile_pool(name="xp", bufs=2))
            tmp = ctx.enter_context(tc.tile_pool(name="tmp", bufs=2))
            ppool = ctx.enter_context(tc.tile_pool(name="pp", bufs=7))
            rcpool = ctx.enter_context(tc.tile_pool(name="rc", bufs=2))
            ypool = ctx.enter_context(tc.tile_pool(name="yp", bufs=6))
            # PSUM: pp(2) + s(2x2) + o(1x2) = 8 banks
            pp_ps = ctx.enter_context(
                tc.tile_pool(name="ppps", bufs=2, space="PSUM")
            )
            spool = ctx.enter_context(
                tc.tile_pool(name="sps", bufs=2, space="PSUM")
            )
            opool = ctx.enter_context(
                tc.tile_pool(name="ops", bufs=1, space="PSUM")
            )

            xrows = {}
            yq = [0]  # alternator for y store queue

            def emit_x_dma(n, engs=(nc.sync, nc.sync)):
                xr = xpool.tile([128, KT, 512], BF16, tag="x", name=f"x{n}")
                xrows[n] = xr
                for half in range(2):
                    ks = slice(4 * half, 4 * half + 4)
                    engs[half].dma_start(
                        xr[:, ks], xt_r[:, ks, bass.ts(n, 512)]
                    )

            def emit_preamble_dma():
                xr = xpool.tile([128, KT, 512], BF16, tag="x", name="x0")
                xrows[0] = xr
                # critical path: proj(0) kv gates on wproj m=0 + x row 0;
                # 2-ktile x chunks let the k-loop start on the first chunk;
                # each queue's order matches first-use order downstream.
                nc.sync.dma_start(wproj_sb[:, 0, 0:4], wproj_d[:, 0, 0:4])
                nc.sync.dma_start(wproj_sb[:, 0, 4:8], wproj_d[:, 0, 4:8])
                nc.sync.dma_start(xr[:, 0:2], xt_r[:, 0:2, bass.ts(0, 512)])
                nc.scalar.dma_start(xr[:, 4:6], xt_r[:, 4:6, bass.ts(0, 512)])
                nc.sync.dma_start(xr[:, 2:4], xt_r[:, 2:4, bass.ts(0, 512)])
                nc.scalar.dma_start(xr[:, 6:8], xt_r[:, 6:8, bass.ts(0, 512)])
                nc.scalar.dma_start(wproj_sb[:, 1], wproj_d[:, 1])
                nc.scalar.dma_start(atab[:, 0:512], atab_d[:, 0:512])
                nc.sync.dma_start(btab[:, 0:512], btab_d[:, 0:512])
                nc.sync.dma_start(rotp[:], rotp_d[:])
                nc.scalar.dma_start(wproj_sb[:, 2], wproj_d[:, 2])
                nc.sync.dma_start(identr[64:128, :], identr_d[:])
                nc.sync.dma_start(trib[:], trib_d[:])
                # zero pads + ones column via gpsimd (keeps DMA queues free)
                nc.gpsimd.memset(kr0[64:128, :], 0.0)
                nc.gpsimd.memset(kr1[0:64, :], 0.0)
                nc.gpsimd.memset(vsb[:, :, 64:128], 1.0)

            def emit_late_consts():
                # issued after proj(0) kv+q01 so row 0's critical chain isn't
                # queued behind these bulk transfers
                emit_x_dma(1, engs=(nc.sync, nc.scalar))
                nc.sync.dma_start(atab[:, 512:T], atab_d[:, 512:T])
                nc.scalar.dma_start(btab[:, 512:T], btab_d[:, 512:T])
                wo_r = wo_d.rearrange("(ko p) c -> p ko c", p=128)
                nc.sync.dma_start(wo_sb[:, 0:1], wo_r[:, 0:1])
                nc.scalar.dma_start(wo_sb[:, 1:2], wo_r[:, 1:2])

            def emit_proj_a(n, m, st):
                """projection phase A for token block n, m-tile m: matmuls,
                PSUM->SBUF copy, rotate-half via PE permutation matmul
                (no DMA on the rope critical path), cos mult.
                m=0: kv (k rope rows 0:64, v transpose), m=1: q01, m=2: q23."""
                xr = xrows[n]
                ps = pp_ps.tile([128, 512], F32, tag="pp", name=f"ps{m}")
                for k in range(KT):
                    nc.tensor.matmul(
                        ps[:],
                        wproj_sb[:, m, k, :],
                        xr[:, k],
                        start=(k == 0),
                        stop=(k == KT - 1),
                    )
                rows = 64 if m == 0 else 128
                qt_t = tmp.tile([128, 512], BF16, tag="qt", name="qt", bufs=3)
                plain = qt_t[:]
                nc.vector.tensor_copy(plain, ps[:])
                qs_ps = pp_ps.tile([128, 512], F32, tag="pp", name=f"qs{m}")
                nc.tensor.matmul(
                    qs_ps[:], rotp[:], plain, start=True, stop=True
                )
                t1 = tmp.tile([128, 512], BF16, tag="t1")
                nc.vector.tensor_tensor(
                    t1[0:rows],
                    plain[0:rows],
                    atab[0:rows, bass.ts(n, 512)],
                    mybir.AluOpType.mult,
                )
                st.update(plain=plain, qs=qs_ps, t1=t1, rows=rows)

            def emit_proj_b(n, m, st):
                """projection phase B: sin mult (reads the permuted copy from
                PSUM) and the rope add."""
                qs, t1, rows = st["qs"], st["t1"], st["rows"]
                t2 = tmp.tile([128, 512], BF16, tag="t2")
                nc.vector.tensor_tensor(
                    t2[0:rows],
                    qs[0:rows],
                    btab[0:rows, bass.ts(n, 512)],
                    mybir.AluOpType.mult,
                )
                dest = kr0 if m == 0 else qr[m - 1]
                nc.vector.tensor_tensor(
                    dest[0:rows, bass.ts(n, 512)],
                    t1[0:rows],
                    t2[0:rows],
                    mybir.AluOpType.add,
                )

            def emit_proj_x(n, m, st):
                """kv extras: kr1 duplicate + v transposes (not needed
                until the first O matmul, so they trail the rope core)."""
                if m != 0:
                    return
                plain = st["plain"]
                nc.vector.tensor_copy(
                    kr1[64:128, bass.ts(n, 512)],
                    kr0[0:64, bass.ts(n, 512)],
                )
                for tt in range(4 * n, 4 * n + 4):
                    vt_ps = spool.tile([128, 64], BF16, tag="s", name="vt")
                    nc.tensor.transpose(
                        vt_ps[:],
                        plain[64:128, bass.ts(tt - 4 * n, 128)],
                        identr[64:128, :],
                    )
                    nc.vector.tensor_copy(vsb[:, tt, 0:64], vt_ps[:])

            def emit_attn_pair(n, hp, fillers=None, pre_norm=None, reserve=3):
                """S/exp/O for one head pair of query row n. After each
                k-block, one filler closure (prev row's Wo pieces / future
                projections) is emitted as ready-to-run PE work to absorb
                exp stalls."""
                fillers = fillers if fillers is not None else []
                jq = n
                nkb = 4 * (jq + 1)
                qtile = qr[hp]
                o_ps = opool.tile([128, 2, 512], F32, tag="o", name=f"o{hp}")
                pends = []

                def emit_o(pnd):
                    kb, col0, p_sb = pnd
                    for hh in range(2):
                        nc.tensor.matmul(
                            o_ps[:, hh, col0:512],
                            vsb[:, kb, :],
                            p_sb[:, hh, col0:512],
                            start=(kb == 0),
                            stop=(kb == nkb - 1),
                        )

                for kb in range(nkb):
                    j = kb - 4 * jq
                    col0 = max(j, 0) * 128
                    s_ps = spool.tile([128, 2, 512], F32, tag="s", name="s")
                    for hh in range(2):
                        krt = kr0 if hh == 0 else kr1
                        nc.tensor.matmul(
                            s_ps[:, hh, col0:512],
                            krt[:, bass.ts(kb, 128)],
                            qtile[:, jq * 512 + col0 : jq * 512 + 512],
                            start=True,
                            stop=True,
                        )
                    p_sb = ppool.tile(
                        [128, 2, 512], BF16, tag="p", name="pb", bufs=7
                    )
                    nc.scalar.activation(
                        p_sb[:, :, col0:512],
                        s_ps[:, :, col0:512],
                        mybir.ActivationFunctionType.Exp,
                        scale=0.125,
                    )
                    if j >= 0:
                        for hh in range(2):
                            nc.gpsimd.tensor_tensor(
                                p_sb[:, hh, col0 : col0 + 128],
                                p_sb[:, hh, col0 : col0 + 128],
                                trib[:],
                                mybir.AluOpType.mult,
                            )
                    # O matmuls run a few k-blocks behind S (p bufs=6) so the
                    # PE queue never reaches an O whose exp hasn't drained.
                    if len(pends) == 4:
                        emit_o(pends.pop(0))
                    pends.append((kb, col0, p_sb))
                    # the deferred previous norm is emitted a few k-blocks in
                    # so its rowsum acts trail this pair's first exps; filler
                    # pops wait for it (they read the opk tile it writes).
                    if pre_norm is not None and kb >= 2:
                        pre_norm()
                        pre_norm = None
                    elif pre_norm is None and len(fillers) > reserve:
                        fillers.pop(0)()
                for pnd in pends:
                    emit_o(pnd)
                if pre_norm is not None:
                    pre_norm()
                # numerators to SBUF (frees the o bank early); rows 64:128
                # hold the rowsum already broadcast across 64 partitions
                # (ones columns of vsb); 1/rowsum via Ln + Exp(-x) on the
                # scalar engine (same act table as the softmax Exp), with Ln
                # reading PSUM directly so it overlaps the staging copy.
                o_sb = rcpool.tile(
                    [64, 2, 512], F32, tag="osb", name=f"osb{jq}_{hp}", bufs=2
                )
                nc.vector.tensor_copy(o_sb[:], o_ps[0:64, :, :])
                lnd = rcpool.tile(
                    [64, 2, 512], F32, tag="lnd", name=f"ln{jq}_{hp}", bufs=2
                )
                nc.scalar.activation(
                    lnd[:], o_ps[64:128, :, :],
                    mybir.ActivationFunctionType.Ln,
                )
                rc = rcpool.tile(
                    [64, 2, 512], F32, tag="rc", name=f"rc{jq}_{hp}", bufs=2
                )
                nc.scalar.activation(
                    rc[:], lnd[:],
                    mybir.ActivationFunctionType.Exp,
                    scale=-1.0,
                )
                return o_sb, rc

            def emit_norm_pair(n, hp, o_sb, rc):
                """apply 1/rowsum via DVE mult -> opk (bf16), all-SBUF."""
                jq = n
                for hh in range(2):
                    nc.vector.tensor_tensor(
                        opk[hp][hh * 64 : hh * 64 + 64, bass.ts(jq, 512)],
                        o_sb[0:64, hh, :],
                        rc[:, hh, :],
                        mybir.AluOpType.mult,
                    )

            def wo_fillers(n, tail_from=None):
                """Wo projection + PSUM->DRAM writeout for row n's 4 token
                blocks, as 8 closures interleaved into the next row's
                attention stream as PE bubble-fill. Pieces from `tail_from`
                onward draw their PSUM bank from the (by then free) o pool
                and copy on the idle Act engine: used for the last row's
                reserved pieces, which run during the final rowsum chain."""
                out = []
                ys = {}

                def piece(t, nn, tailish):
                    def emit():
                        if tailish:
                            wps = opool.tile(
                                [128, 512], F32, tag="o", name="wpso"
                            )
                        else:
                            wps = pp_ps.tile(
                                [128, 512], F32, tag="pp", name="wps"
                            )
                        for k in range(2):
                            nc.tensor.matmul(
                                wps[:],
                                opk[k][:, bass.ts(t, 128)],
                                wo_sb[:, k, bass.ts(nn, 512)],
                                start=(k == 0),
                                stop=(k == 1),
                            )
                        if nn == 0:
                            ys[t] = ypool.tile(
                                [128, C], BF16, tag="y", name="ysb", bufs=4
                            )
                        y_sb = ys[t]
                        if tailish and (yq[0] & 1):
                            nc.scalar.activation(
                                y_sb[:, bass.ts(nn, 512)], wps[:],
                                mybir.ActivationFunctionType.Copy,
                            )
                        else:
                            nc.vector.tensor_copy(
                                y_sb[:, bass.ts(nn, 512)], wps[:]
                            )
                        if tailish:
                            yq[0] += 1
                        if nn == 1:
                            eng = (nc.sync, nc.scalar)[t & 1]
                            eng.dma_start(y_d[bass.ts(t, 128), :], y_sb[:])

                    return emit

                i = 0
                for t in range(4 * n, 4 * n + 4):
                    for nn in range(2):
                        out.append(
                            piece(t, nn, tail_from is not None and i >= tail_from)
                        )
                        i += 1
                return out

            def proj_fillers(n, ms=(0, 1, 2)):
                """A/B/X phases as separate closures, A's leading by one
                slot, so the swap DMA latency hides between filler pops."""
                out = []
                pend = []
                for m in ms:
                    st = {}
                    out.append(
                        (lambda nn, mm, s: lambda: emit_proj_a(nn, mm, s))(n, m, st)
                    )
                    if pend:
                        out.append(pend.pop(0))
                    pend.append(
                        (lambda nn, mm, s: lambda: emit_proj_b(nn, mm, s))(n, m, st)
                    )
                    if m == 0:
                        pend.append(
                            (lambda nn, s: lambda: emit_proj_x(nn, 0, s))(n, st)
                        )
                out.extend(pend)
                return out

            # ---- schedule ----
            # dummy act up front so the act-table load (1.3us) runs during
            # the DMA-bound startup instead of before the first softmax exp
            scr = const.tile([1, 8], F32)
            nc.gpsimd.memset(scr[0:1, 0:4], 1.0)
            nc.scalar.activation(
                scr[0:1, 4:8], scr[0:1, 0:4],
                mybir.ActivationFunctionType.Exp,
            )
            emit_preamble_dma()
            st00, st01 = {}, {}
            emit_proj_a(0, 0, st00)  # kv
            emit_proj_a(0, 1, st01)  # q01
            emit_proj_b(0, 0, st00)
            emit_proj_b(0, 1, st01)
            emit_proj_x(0, 0, st00)
            emit_late_consts()
            # row 0: pair 0 interleaved with the remaining projections
            f0 = proj_fillers(0, (2,)) + proj_fillers(1)
            o_sb, rc = emit_attn_pair(0, 0, f0)
            emit_x_dma(2)
            o_sb1, rc1 = emit_attn_pair(
                0, 1, f0,
                pre_norm=(lambda s, r: lambda: emit_norm_pair(0, 0, s, r))(o_sb, rc),
            )
            while f0:
                f0.pop(0)()
            pending = (lambda s, r: lambda: emit_norm_pair(0, 1, s, r))(o_sb1, rc1)

            for n in range(1, NT):
                fill = wo_fillers(n - 1)
                if n == 1:
                    fill += proj_fillers(2)
                elif n == 2:
                    fill += proj_fillers(3)
                rsv = 6 if n == NT - 1 else 3
                o_sb, rc = emit_attn_pair(n, 0, fill, pre_norm=pending, reserve=rsv)
                if n + 2 < NT:
                    emit_x_dma(n + 2)
                o_sb1, rc1 = emit_attn_pair(
                    n, 1, fill,
                    pre_norm=(lambda nn, s, r: lambda: emit_norm_pair(nn, 0, s, r))(n, o_sb, rc),
                    reserve=rsv,
                )
                while fill:
                    fill.pop(0)()
                pending = (lambda nn, s, r: lambda: emit_norm_pair(nn, 1, s, r))(n, o_sb1, rc1)
            # the last pair's norm gates the tail Wo k=1 accumulation
            pending()
            # tail: the last row's Wo. k=0 contractions could start after
            # norm(3,0), but norm(3,1) only trails by the bc+mult chain; keep
            # the k-split interleave so the k=0 half runs during it.
            n3 = NT - 1
            for t in range(4 * n3, 4 * n3 + 4):
                wpair = []
                for nn in range(2):
                    wpool = pp_ps if (t + nn) % 2 == 0 else spool
                    wps = wpool.tile(
                        [128, 512], F32,
                        tag="pp" if wpool is pp_ps else "s",
                        name="wps",
                    )
                    wpair.append(wps)
                    nc.tensor.matmul(
                        wps[:],
                        opk[0][:, bass.ts(t, 128)],
                        wo_sb[:, 0, bass.ts(nn, 512)],
                        start=True,
                        stop=False,
                    )
                y_sb = ypool.tile([128, C], BF16, tag="y", name="ysb", bufs=4)
                for nn in range(2):
                    wps = wpair[nn]
                    nc.tensor.matmul(
                        wps[:],
                        opk[1][:, bass.ts(t, 128)],
                        wo_sb[:, 1, bass.ts(nn, 512)],
                        start=False,
                        stop=True,
                    )
                    if yq[0] & 1:
                        nc.scalar.activation(
                            y_sb[:, bass.ts(nn, 512)], wps[:],
                            mybir.ActivationFunctionType.Copy,
                        )
                    else:
                        nc.vector.tensor_copy(
                            y_sb[:, bass.ts(nn, 512)], wps[:]
                        )
                    yq[0] += 1
                eng = (nc.sync, nc.scalar)[t & 1]
                eng.dma_start(y_d[bass.ts(t, 128), :], y_sb[:])

    _split_excess_waits(nc)
    return nc


_NC_CACHE = None


def _get_nc():
    global _NC_CACHE
    if _NC_CACHE is None:
        _NC_CACHE = _build()
    return _NC_CACHE


def _host_prep(x, cos, sin, Wq, Wk, Wv, Wo):
    import ml_dtypes

    cos2 = np.asarray(cos, np.float32).reshape(T, HALF)  # [T, 32]
    sin2 = np.asarray(sin, np.float32).reshape(T, HALF)
    atab = np.tile(cos2.T, (4, 1)).astype(ml_dtypes.bfloat16)  # [128, T]
    btab = np.tile(np.vstack([sin2.T, -sin2.T]), (2, 1)).astype(
        ml_dtypes.bfloat16
    )
    k_i = np.arange(128)[:, None]
    q_i = np.arange(128)[None, :]
    trib = np.where(k_i > q_i, np.float32(0.0), np.float32(1.0)).astype(
        ml_dtypes.bfloat16
    )
    identr = np.eye(64, dtype=ml_dtypes.bfloat16)
    ii = np.arange(128)
    rotp = (ii[:, None] == (ii[None, :] ^ 32)).astype(ml_dtypes.bfloat16)

    in_maps = []
    for core in range(8):
        b, g = core // 4, core % 4
        xt = np.ascontiguousarray(
            np.asarray(x[b], np.float32).T.astype(ml_dtypes.bfloat16)
        )  # [C, T]
        # f-tiles ordered kv(128), q01(128), q23(128); laid out
        # [p, m, ko, f] so each m-tile loads as one contiguous-run DMA.
        wfull = np.concatenate(
            [
                Wk[:, g * D : (g + 1) * D],
                Wv[:, g * D : (g + 1) * D],
                Wq[:, g * FQ : (g + 1) * FQ],
            ],
            axis=1,
        ).astype(ml_dtypes.bfloat16)  # [C, 384]: kv | q01 | q23
        wproj = np.ascontiguousarray(
            wfull.reshape(KT, 128, MT, 128).transpose(1, 2, 0, 3)
        )  # [128, MT, KT, 128]
        wo = np.ascontiguousarray(
            Wo[g * FQ : (g + 1) * FQ, :].astype(ml_dtypes.bfloat16)
        )
        in_maps.append(
            {
                "xt": xt,
                "wproj": wproj,
                "wo": wo,
                "atab": atab,
                "btab": btab,
                "trib": trib,
                "identr": identr,
                "rotp": rotp,
            }
        )
    return in_maps


def kernel(x, cos, sin, Wq, Wk, Wv, Wo, _want_trace=False, _trace_kwargs=None):
    nc = _get_nc()
    in_maps = _host_prep(x, cos, sin, Wq, Wk, Wv, Wo)
    kw = {}
    if _want_trace:
        kw = dict(trace=True, **(_trace_kwargs or {}))
    res = run_bass_kernel_spmd(nc, in_maps, list(range(8)), **kw)
    y = np.zeros((B, T, C), np.float32)
    for core in range(8):
        b = core // 4
        y[b] += np.asarray(res.results[core]["y"], dtype=np.float32)
    if _want_trace:
        kernel.last_result = res
    return y


# revision 40
# speedup vs baseline: 1.0345x; 1.0292x over previous
"""Causal self-attention (GQA + RoPE) Trainium2 Bass kernel.

Sharding: 8 cores = batch(2) x kv-group(4). Each core computes its batch's
4 q-heads / 1 kv-head and a row-shard of the Wo projection; the 4 partial
outputs per batch are summed on host (all-reduce replacement).

Fused single-pass pipeline over 512-query rows: for each row n we
project+RoPE x block n, immediately run the causal attention row jq=n
(which only needs k/v blocks 0..4n+3, all available), normalize, and run
the Wo projection + y writeout for the row's 4 token blocks.

Engine budget per core (measured): PE ~120us is the critical engine, so
everything movable is pushed off it and off its feeders:
- exp acts fused across the head pair (one ACTIVATE per k-block).
- softmax denominators via DVE reciprocal_approx_fast (not Act Ln/Exp).
- O normalized straight out of PSUM (no o65 staging copy).
- y written PSUM->DRAM by DMA (no DVE copy, no SBUF staging).
- DMA split across both HW DGE queues (sync + scalar) with kv-first
  weight layout so the first projection unblocks after ~0.8MB.
"""

import numpy as np

import concourse.bass as bass
import concourse.mybir as mybir
from concourse.tile import TileContext
from concourse.bass_utils import run_bass_kernel_spmd

F32 = mybir.dt.float32
F32R = mybir.dt.float32r
BF16 = mybir.dt.bfloat16

B, T, C = 2, 2048, 1024
H, HKV, D = 16, 4, 64
HALF = D // 2  # 32
GQ = H // HKV  # 4 q heads per group
FQ = GQ * D    # 256 q features per group
NT = T // 512  # 4 row blocks of 512
KT = C // 128  # 8 contraction tiles
MT = 3         # m-tiles: 0=kv(128), 1=q01(128), 2=q23(128)


def _split_excess_waits(nc, max_waits=1):
    """walrus here encodes at most one sync-wait per instruction; hoist the
    rest into standalone EventSemaphore instructions (raw-bass encoding)."""
    n = 0
    for fn in nc.m.functions:
        for bb in fn.blocks:
            new = []
            changed = False
            for inst in bb.instructions:
                si = inst.sync_info
                if si is not None and len(si.on_wait) > max_waits:
                    waits = list(si.on_wait)
                    for j, w in enumerate(waits[max_waits:]):
                        ev = mybir.InstEventSemaphore(
                            name=f"{inst.name}-ws{j}",
                            engine=inst.engine,
                            ins=[],
                            outs=[],
                            sync_info=mybir.SyncInfo(on_wait=[w], on_update=[]),
                        )
                        new.append(ev)
                        n += 1
                    inst.sync_info = mybir.SyncInfo(
                        on_wait=waits[:max_waits], on_update=list(si.on_update)
                    )
                    changed = True
                new.append(inst)
            if changed:
                bb.instructions = new
    return n


def _build():
    nc = bass.Bass()
    xt_d = nc.dram_tensor("xt", [C, T], BF16, kind="ExternalInput")
    wproj_d = nc.dram_tensor("wproj", [128, MT, KT, 128], BF16, kind="ExternalInput")
    wo_d = nc.dram_tensor("wo", [FQ, C], BF16, kind="ExternalInput")
    atab_d = nc.dram_tensor("atab", [128, T], BF16, kind="ExternalInput")
    btab_d = nc.dram_tensor("btab", [128, T], BF16, kind="ExternalInput")
    trib_d = nc.dram_tensor("trib", [128, 128], BF16, kind="ExternalInput")
    identr_d = nc.dram_tensor("identr", [64, 64], BF16, kind="ExternalInput")
    rotp_d = nc.dram_tensor("rotp", [128, 128], BF16, kind="ExternalInput")
    y_d = nc.dram_tensor("y", [T, C], BF16, kind="ExternalOutput")

    xt_r = xt_d.rearrange("(ko p) t -> p ko t", p=128)

    with TileContext(nc) as tc:
        from contextlib import ExitStack

        with ExitStack() as ctx:
            const = ctx.enter_context(tc.tile_pool(name="const", bufs=1))
            pers = ctx.enter_context(tc.tile_pool(name="pers", bufs=1))
            # --- constants ---
            wproj_sb = const.tile([128, MT, KT, 128], BF16)
            wo_sb = const.tile([128, 2, C], BF16)
            atab = const.tile([128, T], BF16)
            btab = const.tile([128, T], BF16)
            trib = const.tile([128, 128], BF16)
            identr = const.tile([128, 64], BF16)
            rotp = const.tile([128, 128], BF16)

            # --- persistent activations ---
            qr = [pers.tile([128, T], BF16, name=f"qr{i}") for i in range(2)]
            # k^T zero-padded to 128 contraction rows: kr0 = [k; 0] for even
            # heads, kr1 = [0; k] for odd heads -> S matmuls engage the full
            # PE array while the zero half kills the other head's q rows.
            kr0 = pers.tile([128, T], BF16)
            kr1 = pers.tile([128, T], BF16)
            vsb = pers.tile([128, T // 128, 128], BF16)  # v natural + ones cols
            opk = [pers.tile([128, T], BF16, name=f"opk{i}") for i in range(2)]  # per-pair normalized O^T for Wo

            xpool = ctx.enter_context(tc.tile_pool(name="xp", bufs=2))
            tmp = ctx.enter_context(tc.tile_pool(name="tmp", bufs=2))
            ppool = ctx.enter_context(tc.tile_pool(name="pp", bufs=7))
            rcpool = ctx.enter_context(tc.tile_pool(name="rc", bufs=2))
            ypool = ctx.enter_context(tc.tile_pool(name="yp", bufs=6))
            # PSUM: pp(2) + s(2x2) + o(1x2) = 8 banks
            pp_ps = ctx.enter_context(
                tc.tile_pool(name="ppps", bufs=2, space="PSUM")
            )
            spool = ctx.enter_context(
                tc.tile_pool(name="sps", bufs=2, space="PSUM")
            )
            opool = ctx.enter_context(
                tc.tile_pool(name="ops", bufs=1, space="PSUM")
            )

            xrows = {}
            yq = [0]  # alternator for y store queue

            def emit_x_dma(n, engs=(nc.sync, nc.sync)):
                xr = xpool.tile([128, KT, 512], BF16, tag="x", name=f"x{n}")
                xrows[n] = xr
                for half in range(2):
                    ks = slice(4 * half, 4 * half + 4)
                    engs[half].dma_start(
                        xr[:, ks], xt_r[:, ks, bass.ts(n, 512)]
                    )

            def emit_preamble_dma():
                xr = xpool.tile([128, KT, 512], BF16, tag="x", name="x0")
                xrows[0] = xr
                # critical path: proj(0) kv gates on wproj m=0 + x row 0;
                # 2-ktile x chunks let the k-loop start on the first chunk;
                # each queue's order matches first-use order downstream.
                nc.sync.dma_start(wproj_sb[:, 0, 0:4], wproj_d[:, 0, 0:4])
                nc.sync.dma_start(wproj_sb[:, 0, 4:8], wproj_d[:, 0, 4:8])
                nc.sync.dma_start(xr[:, 0:2], xt_r[:, 0:2, bass.ts(0, 512)])
                nc.scalar.dma_start(xr[:, 4:6], xt_r[:, 4:6, bass.ts(0, 512)])
                nc.sync.dma_start(xr[:, 2:4], xt_r[:, 2:4, bass.ts(0, 512)])
                nc.scalar.dma_start(xr[:, 6:8], xt_r[:, 6:8, bass.ts(0, 512)])
                nc.scalar.dma_start(wproj_sb[:, 1], wproj_d[:, 1])
                nc.scalar.dma_start(atab[:, 0:512], atab_d[:, 0:512])
                nc.sync.dma_start(btab[:, 0:512], btab_d[:, 0:512])
                nc.sync.dma_start(rotp[:], rotp_d[:])
                nc.scalar.dma_start(wproj_sb[:, 2], wproj_d[:, 2])
                nc.sync.dma_start(identr[64:128, :], identr_d[:])
                nc.sync.dma_start(trib[:], trib_d[:])
                # zero pads + ones column via gpsimd (keeps DMA queues free)
                nc.gpsimd.memset(kr0[64:128, :], 0.0)
                nc.gpsimd.memset(kr1[0:64, :], 0.0)
                nc.gpsimd.memset(vsb[:, :, 64:128], 1.0)

            def emit_late_consts():
                # issued after proj(0) kv+q01 so row 0's critical chain isn't
                # queued behind these bulk transfers
                emit_x_dma(1, engs=(nc.sync, nc.scalar))
                nc.sync.dma_start(atab[:, 512:T], atab_d[:, 512:T])
                nc.scalar.dma_start(btab[:, 512:T], btab_d[:, 512:T])
                wo_r = wo_d.rearrange("(ko p) c -> p ko c", p=128)
                nc.sync.dma_start(wo_sb[:, 0:1], wo_r[:, 0:1])
                nc.scalar.dma_start(wo_sb[:, 1:2], wo_r[:, 1:2])

            def emit_proj_a(n, m, st):
                """projection phase A for token block n, m-tile m: matmuls,
                PSUM->SBUF copy, rotate-half via PE permutation matmul
                (no DMA on the rope critical path), cos mult.
                m=0: kv (k rope rows 0:64, v transpose), m=1: q01, m=2: q23."""
                xr = xrows[n]
                ps = pp_ps.tile([128, 512], F32, tag="pp", name=f"ps{m}")
                for k in range(KT):
                    nc.tensor.matmul(
                        ps[:],
                        wproj_sb[:, m, k, :],
                        xr[:, k],
                        start=(k == 0),
                        stop=(k == KT - 1),
                    )
                rows = 64 if m == 0 else 128
                qt_t = tmp.tile([128, 512], BF16, tag="qt", name="qt", bufs=3)
                plain = qt_t[:]
                nc.vector.tensor_copy(plain, ps[:])
                qs_ps = pp_ps.tile([128, 512], F32, tag="pp", name=f"qs{m}")
                nc.tensor.matmul(
                    qs_ps[:], rotp[:], plain, start=True, stop=True
                )
                t1 = tmp.tile([128, 512], BF16, tag="t1")
                nc.vector.tensor_tensor(
                    t1[0:rows],
                    plain[0:rows],
                    atab[0:rows, bass.ts(n, 512)],
                    mybir.AluOpType.mult,
                )
                st.update(plain=plain, qs=qs_ps, t1=t1, rows=rows)

            def emit_proj_b(n, m, st):
                """projection phase B: sin mult (reads the permuted copy from
                PSUM) and the rope add."""
                qs, t1, rows = st["qs"], st["t1"], st["rows"]
                t2 = tmp.tile([128, 512], BF16, tag="t2")
                nc.vector.tensor_tensor(
                    t2[0:rows],
                    qs[0:rows],
                    btab[0:rows, bass.ts(n, 512)],
                    mybir.AluOpType.mult,
                )
                dest = kr0 if m == 0 else qr[m - 1]
                nc.vector.tensor_tensor(
                    dest[0:rows, bass.ts(n, 512)],
                    t1[0:rows],
                    t2[0:rows],
                    mybir.AluOpType.add,
                )

            def emit_proj_x(n, m, st):
                """kv extras: kr1 duplicate + v transposes (not needed
                until the first O matmul, so they trail the rope core)."""
                if m != 0:
                    return
                plain = st["plain"]
                nc.vector.tensor_copy(
                    kr1[64:128, bass.ts(n, 512)],
                    kr0[0:64, bass.ts(n, 512)],
                )
                for tt in range(4 * n, 4 * n + 4):
                    vt_ps = spool.tile([128, 64], BF16, tag="s", name="vt")
                    nc.tensor.transpose(
                        vt_ps[:],
                        plain[64:128, bass.ts(tt - 4 * n, 128)],
                        identr[64:128, :],
                    )
                    nc.vector.tensor_copy(vsb[:, tt, 0:64], vt_ps[:])

            def emit_attn_pair(n, hp, fillers=None, pre_norm=None, reserve=3):
                """S/exp/O for one head pair of query row n. After each
                k-block, one filler closure (prev row's Wo pieces / future
                projections) is emitted as ready-to-run PE work to absorb
                exp stalls."""
                fillers = fillers if fillers is not None else []
                jq = n
                nkb = 4 * (jq + 1)
                qtile = qr[hp]
                o_ps = opool.tile([128, 2, 512], F32, tag="o", name=f"o{hp}")
                pends = []

                def emit_o(pnd):
                    kb, col0, p_sb = pnd
                    for hh in range(2):
                        nc.tensor.matmul(
                            o_ps[:, hh, col0:512],
                            vsb[:, kb, :],
                            p_sb[:, hh, col0:512],
                            start=(kb == 0),
                            stop=(kb == nkb - 1),
                        )

                for kb in range(nkb):
                    j = kb - 4 * jq
                    col0 = max(j, 0) * 128
                    s_ps = spool.tile([128, 2, 512], F32, tag="s", name="s")
                    for hh in range(2):
                        krt = kr0 if hh == 0 else kr1
                        nc.tensor.matmul(
                            s_ps[:, hh, col0:512],
                            krt[:, bass.ts(kb, 128)],
                            qtile[:, jq * 512 + col0 : jq * 512 + 512],
                            start=True,
                            stop=True,
                        )
                    p_sb = ppool.tile(
                        [128, 2, 512], BF16, tag="p", name="pb", bufs=7
                    )
                    nc.scalar.activation(
                        p_sb[:, :, col0:512],
                        s_ps[:, :, col0:512],
                        mybir.ActivationFunctionType.Exp,
                        scale=0.125,
                    )
                    if j >= 0:
                        for hh in range(2):
                            nc.gpsimd.tensor_tensor(
                                p_sb[:, hh, col0 : col0 + 128],
                                p_sb[:, hh, col0 : col0 + 128],
                                trib[:],
                                mybir.AluOpType.mult,
                            )
                    # O matmuls run a few k-blocks behind S (p bufs=6) so the
                    # PE queue never reaches an O whose exp hasn't drained.
                    if len(pends) == 4:
                        emit_o(pends.pop(0))
                    pends.append((kb, col0, p_sb))
                    # the deferred previous norm is emitted a few k-blocks in
                    # so its rowsum acts trail this pair's first exps; filler
                    # pops wait for it (they read the opk tile it writes).
                    if pre_norm is not None and kb >= 2:
                        pre_norm()
                        pre_norm = None
                    elif pre_norm is None and len(fillers) > reserve:
                        fillers.pop(0)()
                for pnd in pends:
                    emit_o(pnd)
                if pre_norm is not None:
                    pre_norm()
                # rows 64:128 hold the rowsum already broadcast across 64
                # partitions (ones columns of vsb); 1/rowsum via Ln + Exp(-x)
                # on the scalar engine (same act table as the softmax Exp).
                # The numerators stay in PSUM: the deferred norm mult reads
                # them there directly (one PSUM input is allowed), so no
                # staging copy is needed.
                lnd = rcpool.tile(
                    [64, 2, 512], F32, tag="lnd", name=f"ln{jq}_{hp}", bufs=2
                )
                nc.scalar.activation(
                    lnd[:], o_ps[64:128, :, :],
                    mybir.ActivationFunctionType.Ln,
                )
                rc = rcpool.tile(
                    [64, 2, 512], F32, tag="rc", name=f"rc{jq}_{hp}", bufs=2
                )
                nc.scalar.activation(
                    rc[:], lnd[:],
                    mybir.ActivationFunctionType.Exp,
                    scale=-1.0,
                )
                return o_ps, rc

            def emit_norm_pair(n, hp, o_ps, rc):
                """apply 1/rowsum via DVE mult -> opk (bf16), numerators
                read straight from the PSUM O accumulator."""
                jq = n
                for hh in range(2):
                    nc.vector.tensor_tensor(
                        opk[hp][hh * 64 : hh * 64 + 64, bass.ts(jq, 512)],
                        o_ps[0:64, hh, :],
                        rc[:, hh, :],
                        mybir.AluOpType.mult,
                    )

            def wo_fillers(n, tail_from=None):
                """Wo projection + PSUM->DRAM writeout for row n's 4 token
                blocks, as 8 closures interleaved into the next row's
                attention stream as PE bubble-fill. Pieces from `tail_from`
                onward draw their PSUM bank from the (by then free) o pool
                and copy on the idle Act engine: used for the last row's
                reserved pieces, which run during the final rowsum chain."""
                out = []
                ys = {}

                def piece(t, nn, tailish):
                    def emit():
                        if tailish:
                            wps = opool.tile(
                                [128, 512], F32, tag="o", name="wpso"
                            )
                        else:
                            wps = pp_ps.tile(
                                [128, 512], F32, tag="pp", name="wps"
                            )
                        for k in range(2):
                            nc.tensor.matmul(
                                wps[:],
                                opk[k][:, bass.ts(t, 128)],
                                wo_sb[:, k, bass.ts(nn, 512)],
                                start=(k == 0),
                                stop=(k == 1),
                            )
                        if nn == 0:
                            ys[t] = ypool.tile(
                                [128, C], BF16, tag="y", name="ysb", bufs=4
                            )
                        y_sb = ys[t]
                        if tailish and (yq[0] & 1):
                            nc.scalar.activation(
                                y_sb[:, bass.ts(nn, 512)], wps[:],
                                mybir.ActivationFunctionType.Copy,
                            )
                        else:
                            nc.vector.tensor_copy(
                                y_sb[:, bass.ts(nn, 512)], wps[:]
                            )
                        if tailish:
                            yq[0] += 1
                        if nn == 1:
                            eng = (nc.sync, nc.scalar)[t & 1]
                            eng.dma_start(y_d[bass.ts(t, 128), :], y_sb[:])

                    return emit

                i = 0
                for t in range(4 * n, 4 * n + 4):
                    for nn in range(2):
                        out.append(
                            piece(t, nn, tail_from is not None and i >= tail_from)
                        )
                        i += 1
                return out

            def proj_fillers(n, ms=(0, 1, 2)):
                """A/B/X phases as separate closures, A's leading by one
                slot, so the swap DMA latency hides between filler pops."""
                out = []
                pend = []
                for m in ms:
                    st = {}
                    out.append(
                        (lambda nn, mm, s: lambda: emit_proj_a(nn, mm, s))(n, m, st)
                    )
                    if pend:
                        out.append(pend.pop(0))
                    pend.append(
                        (lambda nn, mm, s: lambda: emit_proj_b(nn, mm, s))(n, m, st)
                    )
                    if m == 0:
                        pend.append(
                            (lambda nn, s: lambda: emit_proj_x(nn, 0, s))(n, st)
                        )
                out.extend(pend)
                return out

            # ---- schedule ----
            # dummy act up front so the act-table load (1.3us) runs during
            # the DMA-bound startup instead of before the first softmax exp
            scr = const.tile([1, 8], F32)
            nc.gpsimd.memset(scr[0:1, 0:4], 1.0)
            nc.scalar.activation(
                scr[0:1, 4:8], scr[0:1, 0:4],
                mybir.ActivationFunctionType.Exp,
            )
            emit_preamble_dma()
            st00, st01 = {}, {}
            emit_proj_a(0, 0, st00)  # kv
            emit_proj_a(0, 1, st01)  # q01
            emit_proj_b(0, 0, st00)
            emit_proj_b(0, 1, st01)
            emit_proj_x(0, 0, st00)
            emit_late_consts()
            # row 0: pair 0 interleaved with the remaining projections
            f0 = proj_fillers(0, (2,)) + proj_fillers(1)
            o_sb, rc = emit_attn_pair(0, 0, f0)
            emit_x_dma(2)
            o_sb1, rc1 = emit_attn_pair(
                0, 1, f0,
                pre_norm=(lambda s, r: lambda: emit_norm_pair(0, 0, s, r))(o_sb, rc),
            )
            while f0:
                f0.pop(0)()
            pending = (lambda s, r: lambda: emit_norm_pair(0, 1, s, r))(o_sb1, rc1)

            for n in range(1, NT):
                fill = wo_fillers(n - 1)
                if n == 1:
                    fill += proj_fillers(2)
                elif n == 2:
                    fill += proj_fillers(3)
                rsv = 6 if n == NT - 1 else 3
                o_sb, rc = emit_attn_pair(n, 0, fill, pre_norm=pending, reserve=rsv)
                if n + 2 < NT:
                    emit_x_dma(n + 2)
                o_sb1, rc1 = emit_attn_pair(
                    n, 1, fill,
                    pre_norm=(lambda nn, s, r: lambda: emit_norm_pair(nn, 0, s, r))(n, o_sb, rc),
                    reserve=rsv,
                )
                while fill:
                    fill.pop(0)()
                pending = (lambda nn, s, r: lambda: emit_norm_pair(nn, 1, s, r))(n, o_sb1, rc1)
            # the last pair's norm gates the tail Wo k=1 accumulation
            pending()
            # tail: the last row's Wo. k=0 contractions could start after
            # norm(3,0), but norm(3,1) only trails by the bc+mult chain; keep
            # the k-split interleave so the k=0 half runs during it.
            n3 = NT - 1
            for t in range(4 * n3, 4 * n3 + 4):
                wpair = []
                for nn in range(2):
                    wpool = pp_ps if (t + nn) % 2 == 0 else spool
                    wps = wpool.tile(
                        [128, 512], F32,
                        tag="pp" if wpool is pp_ps else "s",
                        name="wps",
                    )
                    wpair.append(wps)
                    nc.tensor.matmul(
                        wps[:],
                        opk[0][:, bass.ts(t, 128)],
                        wo_sb[:, 0, bass.ts(nn, 512)],
                        start=True,
                        stop=False,
                    )
                y_sb = ypool.tile([128, C], BF16, tag="y", name="ysb", bufs=4)
                for nn in range(2):
                    wps = wpair[nn]
                    nc.tensor.matmul(
                        wps[:],
                        opk[1][:, bass.ts(t, 128)],
                        wo_sb[:, 1, bass.ts(nn, 512)],
                        start=False,
                        stop=True,
                    )
                    if yq[0] & 1:
                        nc.scalar.activation(
                            y_sb[:, bass.ts(nn, 512)], wps[:],
                            mybir.ActivationFunctionType.Copy,
                        )
                    else:
                        nc.vector.tensor_copy(
                            y_sb[:, bass.ts(nn, 512)], wps[:]
                        )
                    yq[0] += 1
                eng = (nc.sync, nc.scalar)[t & 1]
                eng.dma_start(y_d[bass.ts(t, 128), :], y_sb[:])

    _split_excess_waits(nc)
    return nc


_NC_CACHE = None


def _get_nc():
    global _NC_CACHE
    if _NC_CACHE is None:
        _NC_CACHE = _build()
    return _NC_CACHE


def _host_prep(x, cos, sin, Wq, Wk, Wv, Wo):
    import ml_dtypes

    cos2 = np.asarray(cos, np.float32).reshape(T, HALF)  # [T, 32]
    sin2 = np.asarray(sin, np.float32).reshape(T, HALF)
    atab = np.tile(cos2.T, (4, 1)).astype(ml_dtypes.bfloat16)  # [128, T]
    btab = np.tile(np.vstack([sin2.T, -sin2.T]), (2, 1)).astype(
        ml_dtypes.bfloat16
    )
    k_i = np.arange(128)[:, None]
    q_i = np.arange(128)[None, :]
    trib = np.where(k_i > q_i, np.float32(0.0), np.float32(1.0)).astype(
        ml_dtypes.bfloat16
    )
    identr = np.eye(64, dtype=ml_dtypes.bfloat16)
    ii = np.arange(128)
    rotp = (ii[:, None] == (ii[None, :] ^ 32)).astype(ml_dtypes.bfloat16)

    in_maps = []
    for core in range(8):
        b, g = core // 4, core % 4
        xt = np.ascontiguousarray(
            np.asarray(x[b], np.float32).T.astype(ml_dtypes.bfloat16)
        )  # [C, T]
        # f-tiles ordered kv(128), q01(128), q23(128); laid out
        # [p, m, ko, f] so each m-tile loads as one contiguous-run DMA.
        wfull = np.concatenate(
            [
                Wk[:, g * D : (g + 1) * D],
                Wv[:, g * D : (g + 1) * D],
                Wq[:, g * FQ : (g + 1) * FQ],
            ],
            axis=1,
        ).astype(ml_dtypes.bfloat16)  # [C, 384]: kv | q01 | q23
        wproj = np.ascontiguousarray(
            wfull.reshape(KT, 128, MT, 128).transpose(1, 2, 0, 3)
        )  # [128, MT, KT, 128]
        wo = np.ascontiguousarray(
            Wo[g * FQ : (g + 1) * FQ, :].astype(ml_dtypes.bfloat16)
        )
        in_maps.append(
            {
                "xt": xt,
                "wproj": wproj,
                "wo": wo,
                "atab": atab,
                "btab": btab,
                "trib": trib,
                "identr": identr,
                "rotp": rotp,
            }
        )
    return in_maps


def kernel(x, cos, sin, Wq, Wk, Wv, Wo, _want_trace=False, _trace_kwargs=None):
    nc = _get_nc()
    in_maps = _host_prep(x, cos, sin, Wq, Wk, Wv, Wo)
    kw = {}
    if _want_trace:
        kw = dict(trace=True, **(_trace_kwargs or {}))
    res = run_bass_kernel_spmd(nc, in_maps, list(range(8)), **kw)
    y = np.zeros((B, T, C), np.float32)
    for core in range(8):
        b = core // 4
        y[b] += np.asarray(res.results[core]["y"], dtype=np.float32)
    if _want_trace:
        kernel.last_result = res
    return y
